# revision 4
# baseline (speedup 1.0000x reference)
"""Trainium2 Bass kernel for nn_AttentionBlock (sparse causal attention).

Math (per batch b, head h), A = r_prime[b] (T x N):
    out[b] = sum_h tril(A Q_h A^T) @ (A E_h^T)
Data-parallel over batch (8 batches -> 8 NeuronCores); per core a chunked
(C=128) linear-attention decomposition in bf16 with fp32 PSUM.

Key facts validated on this runtime (quirk_test.py) that differ from the
previous session's assumptions:
- matmul operands MAY live at SBUF partition base 64 (lhsT and rhs must
  share the base), so the odd-head half of C = (A Q)^T is read in place
  against a copy of rpt duplicated on partitions 64..127 (the old c_hi
  relocation DMA is gone).
- matmul MAY write PSUM at partition offset 64, so the running state is
  accumulated parity-stacked ([j of even heads; j of odd heads] x
  (pair, i)), which halves the snapshot copy (one [128,256] DVE copy)
  and halves the inter MM count (4 pair-stacked K=128 MMs per chunk).
- matmul PSUM output must be fp32 (bf16 PSUM rejected by bass).

Per chunk: W = A_I C for all 8 heads into one [128,1024] fp32 PSUM pair
(double-buffered, prefetched one chunk ahead), wm = W*tril-mask as a
single DVE tensor_mul, out[t,i] += sum_h wm_h^T er_h (8 N=64 MMs, FWL
weight loads) + sum_pair C_pair^T S_pair (4 MMs); state P += A_I^T Er_I
(2 parity MMs). Engine balance: DVE does mask+state snapshot, ACT does
C/Er/out evacuations. Output is accumulated [t, i]-major in one PSUM
bank per 8 chunks, so the host does a cheap reshape (no transpose).
For_i timing loops unroll 4 reps per iteration (all-engine loop barrier
amortized; larger bodies regress).
"""

import numpy as np

import concourse.bacc as bacc
import concourse.bass as bass
import concourse.mybir as mybir
import concourse.tile as tile
from concourse.bass_utils import run_bass_kernel_spmd

D, T, N, H = 8, 2048, 64, 8
C = 128
BF = mybir.dt.bfloat16
F8 = mybir.dt.float8e4
F32 = mybir.dt.float32
NP_BF = mybir.dt.np(BF)


def build_nc(t_len: int = T, reps: int = 1, loop_reps: int = 1,
             unroll: int = 0, s16alt: int = 0, stream: int = 0,
             erfirst: int = 0, hints: int = 1, cpre: int = 2,
             wm8: int = 0, obf: int = 0) -> bacc.Bacc:
    nch = t_len // C
    assert nch % 2 == 0
    if unroll == 0:
        unroll = 4 if loop_reps % 4 == 0 else 1
    nc = bacc.Bacc("TRN2", target_bir_lowering=False, debug=False)

    rp = nc.dram_tensor("rp", [C, nch * N], BF, kind="ExternalInput")
    rpt = nc.dram_tensor("rpt", [N, t_len], BF, kind="ExternalInput")
    q_all = nc.dram_tensor("q_all", [N, H * N], BF, kind="ExternalInput")
    et_all = nc.dram_tensor("et_all", [N, H * N], BF, kind="ExternalInput")
    maskd = nc.dram_tensor("maskd", [C, 8 * C], BF, kind="ExternalInput")
    out_t = nc.dram_tensor("out_t", [C, nch * N],
                           mybir.dt.bfloat16 if obf else F32,
                           kind="ExternalOutput")

    with tile.TileContext(nc) as tc:
        with (
            tc.tile_pool(name="const", bufs=1) as cpool,
            tc.tile_pool(name="csb", bufs=4) as c_pool,
            tc.tile_pool(name="ersb", bufs=4) as er_pool,
            tc.tile_pool(name="wm", bufs=4) as wm_pool,
            tc.tile_pool(name="s16p", bufs=3) as s16_pool,
            tc.tile_pool(name="ps_c", bufs=1, space="PSUM") as ps_c,
            tc.tile_pool(name="ps_w", bufs=2, space="PSUM") as ps_w,
            tc.tile_pool(name="ps_er", bufs=1, space="PSUM") as ps_er,
            tc.tile_pool(name="ps_s", bufs=1, space="PSUM") as ps_s,
            tc.tile_pool(name="ps_o", bufs=1, space="PSUM") as ps_o,
        ):
            q_sb = cpool.tile([N, H * N], BF)
            nc.gpsimd.dma_start(q_sb[:], q_all[:])
            et_sb = cpool.tile([N, H * N], BF)
            nc.gpsimd.dma_start(et_sb[:], et_all[:])
            # rpt duplicated on both partition halves: base-64 operands for
            # the odd-head W matmuls
            rpt_sb = cpool.tile([2 * N, t_len], BF)
            for pc in range(4):
                sl = slice(pc * t_len // 4, (pc + 1) * t_len // 4)
                nc.sync.dma_start(rpt_sb[0:N, sl], rpt[:, sl])
                nc.sync.dma_start(rpt_sb[N : 2 * N, sl], rpt[:, sl])
            rp_sb = cpool.tile([C, nch * N], BF)
            for pc in range(2):
                sl = slice(pc * nch * N // 2, (pc + 1) * nch * N // 2)
                nc.scalar.dma_start(rp_sb[:, sl], rp[:, sl])
            mask_sb = cpool.tile([C, 8 * C], BF)
            nc.gpsimd.dma_start(mask_sb[:], maskd[:])
            out_sb = cpool.tile([C, nch * N], BF if obf else F32)
            scr = cpool.tile([1, 4], BF)
            nc.vector.tensor_copy(scr[:], mask_sb[:1, :4])

            def make_stream():
                state = {"p_s": None, "p_o2": None}
                c_tiles = {}
                er_tiles = {}
                wm_tiles = {}

                def emit_c(u, ii):
                    # C for both chunks of pair ii, all 8 heads:
                    # c2_sb[64e+k, 256p+128m+t] = C_{2p+e}[k, t(of chunk m)]
                    psl = slice(ii * 2 * C, (ii + 1) * 2 * C)
                    c2_sb = c_pool.tile([2 * N, 8 * C], BF, tag="c2")
                    for q in range(2):
                        p_c = ps_c.tile([2 * N, 4 * C], F32, tag="c")
                        for pp in range(2):
                            p = 2 * q + pp
                            nc.tensor.matmul(
                                p_c[:, pp * 2 * C : (pp + 1) * 2 * C],
                                lhsT=q_sb[:, p * 2 * N : (p + 1) * 2 * N],
                                rhs=rpt_sb[0:N, psl],
                                start=(pp == 0),
                                stop=(pp == 1),
                            )
                        nc.scalar.copy(
                            c2_sb[:, q * 4 * C : (q + 1) * 4 * C], p_c[:]
                        )
                    c_tiles[(u, ii)] = c2_sb

                def emit_er(u, i):
                    tsl = slice(i * C, (i + 1) * C)
                    p_er = ps_er.tile([C, H * N], F32, tag="er")
                    nc.tensor.matmul(
                        p_er[:], lhsT=rpt_sb[0:N, tsl], rhs=et_sb[:],
                        start=True, stop=True,
                    )
                    er_sb = er_pool.tile([C, H * N], BF, tag="er_sb")
                    nc.scalar.copy(er_sb[:], p_er[:])
                    er_tiles[(u, i)] = er_sb

                def emit_w(u, i):
                    # W+mask for chunk i, all heads; odd heads read C's hi
                    # half via base-64 operands
                    ii, m = i // 2, i % 2
                    c2_sb = c_tiles[(u, ii)]
                    tsl = slice(i * C, (i + 1) * C)
                    p_w = ps_w.tile([C, 8 * C], F32, tag="w")
                    for e in range(2):
                        csrc = c2_sb[0:N, :] if e == 0 else c2_sb[N : 2 * N, :]
                        c_v = csrc.rearrange(
                            "k (p mm t) -> k p mm t", p=4, mm=2
                        )
                        nc.tensor.matmul(
                            p_w[:, e * 4 * C : (e + 1) * 4 * C],
                            lhsT=rpt_sb[e * N : (e + 1) * N, tsl],
                            rhs=c_v[:, :, m, :],
                            start=True,
                            stop=True,
                        )
                    wm = wm_pool.tile([C, 8 * C], F8 if wm8 else BF,
                                      tag="wm")
                    nc.vector.tensor_mul(wm[:], p_w[:], mask_sb[:])
                    wm_tiles[(u, i)] = wm

                def preamble(u):
                    for jj in range(min(cpre, nch // 2)):
                        emit_c(u, jj)
                    emit_er(u, 0)
                    emit_w(u, 0)

                def body(u, last):
                    # one rep's 16 chunks; unless `last`, the next rep's
                    # preamble is emitted during the final chunk
                    for ii in range(nch // 2):
                        c2_sb = c_tiles[(u, ii)]
                        for m in range(2):
                            i = 2 * ii + m
                            chunk(u, ii, m, i, c2_sb)
                            if stream and i == nch - 2 and not last:
                                preamble(u + 1)

                def chunk(u, ii, m, i, c2_sb):
                    if True:
                        # parity-stacked state snapshot (before P update)
                        if i > 0:
                            p_s = state["p_s"]
                            s16f = s16_pool.tile(
                                [2 * N, 4 * N], BF, tag="s16f"
                            )
                            if s16alt and i % 2 == 1:
                                nc.scalar.copy(s16f[:], p_s[:])
                            else:
                                nc.vector.tensor_copy(s16f[:], p_s[:])
                        if i + 1 < nch:
                            emit_w(u, i + 1)
                            if erfirst:
                                emit_er(u, i + 1)
                        if m == 0 and ii + cpre < nch // 2:
                            emit_c(u, ii + cpre)
                        if i + 1 < nch and not erfirst:
                            emit_er(u, i + 1)
                        er_sb = er_tiles.pop((u, i))
                        wm = wm_tiles.pop((u, i))

                        s = i % 8
                        if s == 0:
                            p_o2 = ps_o.tile([C, 8 * N], F32, tag="o")
                            state["p_o2"] = p_o2
                        p_o2 = state["p_o2"]
                        p_o = p_o2[:, s * N : (s + 1) * N]
                        n_mm = 8 if i == 0 else 12
                        g = 0
                        for h in (0, 2, 4, 6, 1, 3, 5, 7):
                            e, gh = h % 2, h // 2
                            ge = e * 4 + gh
                            nc.tensor.matmul(
                                p_o,
                                lhsT=wm[:, (e * 4 + gh) * C :
                                        (e * 4 + gh + 1) * C],
                                rhs=er_sb[:, ge * N : (ge + 1) * N],
                                start=(g == 0),
                                stop=(g == n_mm - 1),
                                skip_group_check=True,
                            )
                            g += 1
                        if i > 0:
                            # inter: pair-stacked K=128 against the
                            # parity-stacked snapshot
                            for p in range(4):
                                nc.tensor.matmul(
                                    p_o,
                                    lhsT=c2_sb[:, p * 2 * C + m * C :
                                               p * 2 * C + (m + 1) * C],
                                    rhs=s16f[:, p * N : (p + 1) * N],
                                    start=False,
                                    stop=(g == n_mm - 1),
                                    skip_group_check=True,
                                )
                                g += 1
                        if s == 7:
                            w8 = i // 8
                            osl = slice(w8 * 8 * N, (w8 + 1) * 8 * N)
                            nc.scalar.copy(out_sb[:, osl], p_o2[:])
                            nc.sync.dma_start(out_t[:, osl], out_sb[:, osl])

                        # state update, parity-stacked: lo partitions get
                        # even heads' P, hi partitions odd heads'
                        if i < nch - 1:
                            if i == 0:
                                p_s_new = ps_s.tile([2 * N, 4 * N], F32,
                                                    tag="s")
                                state["p_s"] = p_s_new
                            p_s = state["p_s"]
                            for e in range(2):
                                nc.tensor.matmul(
                                    p_s[e * N : (e + 1) * N, :],
                                    lhsT=rp_sb[:, i * N : (i + 1) * N],
                                    rhs=er_sb[:, e * 4 * N : (e + 1) * 4 * N],
                                    start=(i == 0),
                                    stop=(i == nch - 2),
                                    skip_group_check=True,
                                )

                return preamble, body

            preamble, body = make_stream()

            def run_group(n):
                if stream:
                    preamble(0)
                    for u in range(n):
                        body(u, last=(u == n - 1))
                else:
                    for u in range(n):
                        preamble(u)
                        body(u, last=True)

            if loop_reps > 1:
                assert loop_reps % unroll == 0
                hint_sets = {
                    0: (),
                    1: (mybir.EngineType.PE, mybir.EngineType.Activation,
                        mybir.EngineType.DVE, mybir.EngineType.SP),
                    2: (mybir.EngineType.PE, mybir.EngineType.Activation,
                        mybir.EngineType.DVE, mybir.EngineType.SP,
                        mybir.EngineType.Pool),
                }
                with tc.For_i(
                    0, loop_reps // unroll, 1,
                    hint_engines=hint_sets[hints],
                ):
                    run_group(unroll)
            else:
                run_group(reps)

    nc.compile()
    return nc


def _host_prep(r_prime: np.ndarray, Q: np.ndarray, E: np.ndarray, t_len: int = T):
    nch = t_len // C
    q_all = np.ascontiguousarray(
        Q.transpose(1, 0, 2).reshape(N, H * N)
    ).astype(NP_BF)
    perm = [0, 2, 4, 6, 1, 3, 5, 7]
    et_all = np.ascontiguousarray(
        E[perm].transpose(2, 0, 1).reshape(N, H * N)
    ).astype(NP_BF)
    mask = np.tile(np.triu(np.ones((C, C), np.float32)), (1, 8)).astype(NP_BF)
    in_maps = []
    for b in range(D):
        a = r_prime[b]
        rp16 = (
            a.reshape(nch, C, N).transpose(1, 0, 2).reshape(C, nch * N)
        ).astype(NP_BF)
        rpt16 = np.ascontiguousarray(a.T).astype(NP_BF)
        in_maps.append(
            {
                "rp": rp16,
                "rpt": rpt16,
                "q_all": q_all,
                "et_all": et_all,
                "maskd": mask,
            }
        )
    return in_maps


def _unshard(res, t_len: int = T):
    nch = t_len // C
    outs = []
    for b in range(D):
        o = np.asarray(res[b]["out_t"], np.float32)
        outs.append(
            o.reshape(C, nch, N).transpose(1, 0, 2).reshape(t_len, N)
        )
    return np.stack(outs).astype(np.float32)


_NC_CACHE: dict = {}


def kernel(r_prime: np.ndarray, Q: np.ndarray, E: np.ndarray) -> np.ndarray:
    r_prime = np.asarray(r_prime, np.float32)
    Q = np.asarray(Q, np.float32)
    E = np.asarray(E, np.float32)
    t_len = r_prime.shape[1]
    if ("nc", t_len) not in _NC_CACHE:
        _NC_CACHE[("nc", t_len)] = build_nc(t_len)
    nc = _NC_CACHE[("nc", t_len)]
    in_maps = _host_prep(r_prime, Q, E, t_len)
    res = run_bass_kernel_spmd(nc, in_maps, list(range(D)))
    return _unshard([res.results[b] for b in range(D)], t_len)
